# revision 22
# baseline (speedup 1.0000x reference)
"""MetaPathGNN forward on 8 Trainium2 NeuronCores (Bass/Tile).

Strategy (self-contained; shapes hardcoded for N=100000, C=256, OUT=128, E=400000):
  - Nodes sharded 12500/core. Per layer: each core computes hw = h @ wlT for its
    nodes (fp16), staged in an SBUF slab per chunk of 25 node-tiles, then
    AllGathered chunk-by-chunk (4 chunks) so the collective streams behind
    compute. Gather windows == chunk outputs (<=25600 rows, int16-addressable).
  - Edges assigned to cores by src owner; host sorts each core's edges by
    (super-tile(src), window(dst), tile(src)) and pads so the slot layout
    is identical across cores (single SPMD NEFF). The guaranteed arange
    self-edge of every node is handled by one identity matmul per tile fed
    from a contiguous reload of the core's own hw rows (removes 12.5k random
    gather slots per layer); random self-edges stay in slots.
  - Messages gathered with GpSimd dma_gather (512B fp16 rows, int16 indices
    against per-chunk tables). xT dense lhs loaded per 25-tile chunk in one
    1.6MB DMA instead of 98 64KB DMAs.
  - Segment-sum = fp8(0/1 selector) x fp16(messages) matmuls accumulated in
    PSUM, one [128,512] bank per pair of 128-node tiles.
  - Epilogue fuses deg-normalize + dense term, relu + LN stats, normalize;
    LN stats computed per super-tile so downstream work pipelines.
  - Layer-2 dense matmuls + its chunked AllGather are interleaved into
    layer-1's gather loop; the final projection is interleaved into layer-2's
    gather loop, with outputs DMAed straight from PSUM.
"""
import numpy as np
from contextlib import ExitStack

N = 100000
C = 256
OUT = 128
NCORES = 8
NPC = N // NCORES          # 12500 nodes per core
P = 128
TILES = (NPC + P - 1) // P  # 98
NPC_PAD = TILES * P         # 12544
ST_TILES = 8                # node-tiles per super-tile
NST = (TILES + ST_TILES - 1) // ST_TILES  # 13
LN_EPS = 1e-5

# AllGather chunking: 4 chunks of node-tiles per core; window w == chunk w.
CHUNK_TILES = [25, 25, 25, 23]
CHUNK_T0 = [0, 25, 50, 75]
CHUNK_ROWS = [3200, 3200, 3200, 2900]   # real (unpadded) rows per core
CHUNK_BASE_R = [0, 3200, 6400, 9600]
NWIN = 4
NWIN_G = 5  # gather windows: 0 = local (own ag bounce), 1..4 = table windows
WIN_ROWS = [r * NCORES for r in CHUNK_ROWS]  # [25600,25600,25600,23200]

import os as _os
INTERLEAVE = _os.environ.get("K_INTERLEAVE", "1") == "1"
# NCHUNK=4: per-chunk tables + 4 AllGathers/layer. NCHUNK=1: one table +
# one AllGather/layer with overlapping 32768-row windows (baseline scheme).
NCHUNK = int(_os.environ.get("K_NCHUNK", "4"))
# FP8 message tables: hw staged/gathered/exchanged as float8e4 instead of
# fp16 — halves AllGather wire bytes and gather DMA traffic.
FP8 = _os.environ.get("K_FP8", "0") == "1"  # fails 2e-2 gate (3.7e-2): deg-1
# nodes take a full single-row fp8 quantization hit; keep fp16 tables.
REPEAT = int(_os.environ.get("K_REPEAT", "1"))  # bench: repeat pipeline in-NEFF
NOAG = _os.environ.get("K_NOAG", "0") == "1"     # bench: skip collectives (wrong results)
NOGATHER = _os.environ.get("K_NOGATHER", "0") == "1"  # bench: skip dma_gathers
NOSEL = _os.environ.get("K_NOSEL", "0") == "1"   # bench: skip selector matmuls
WIN = 25000  # window stride in single-table mode

_COMPILED = {}


# ---------------------------------------------------------------- host side
def _sigmoid(x):
    return 1.0 / (1.0 + np.exp(-np.float64(x)))


def _build_layer(src, dst):
    """Vectorized layout builder. Returns dict with:
       structure: per st -> list of (w, n_slots, mm_list(block,tile), slot_base, mm_base)
       idx:  [NCORES, S] int16 window-local gather indices
       sel:  [NCORES, 128, NMM*128] 0/1 fp8 selector blob
    """
    rows_w = np.asarray(CHUNK_ROWS, dtype=np.int64)
    base_w = np.asarray(CHUNK_BASE_R, dtype=np.int64)
    per_core = []
    for c in range(NCORES):
        lo = c * NPC
        m = (src >= lo) & (src < lo + NPC)
        s = (src[m] - lo).astype(np.int64)
        dg = dst[m].astype(np.int64)
        # window 0 = LOCAL: dst owned by this core (incl. all self-edges),
        # gathered from the core's own ag bounce buffer -- uniform across
        # cores and independent of the collective. windows 1..4 = table.
        is_local = (dg >= lo) & (dg < lo + NPC)
        if NCHUNK == 1:
            # single node-major table; overlapping windows of stride WIN
            wt = np.minimum(dg // WIN, 3)
            dloc_t = dg - wt * WIN
        else:
            dc = dg // NPC
            dr = dg - dc * NPC
            wt = np.minimum(dr // 3200, 3)
            dloc_t = dc * rows_w[wt] + (dr - base_w[wt])
        w = np.where(is_local, 0, wt + 1)
        dloc = np.where(is_local, dg - lo, dloc_t)
        t = s >> 7
        sti = t // ST_TILES
        order = np.lexsort((s, t, w, sti))
        per_core.append((s[order], dloc[order], w[order], t[order], sti[order]))

    cnt = np.zeros((NCORES, NST, NWIN_G, TILES), dtype=np.int64)
    for c in range(NCORES):
        s, d, w, t, sti = per_core[c]
        np.add.at(cnt[c], (sti, w, t), 1)
    ucnt = cnt.max(axis=0)

    structure = []
    total_slots = 0
    total_mms = 0
    slot_tile_all = []
    seg_start = {}  # (st,w,tile) -> unified slot start position
    for sti in range(NST):
        st_runs = []
        for wi in range(NWIN_G):
            segs = [(ti, int(ucnt[sti, wi, ti]))
                    for ti in range(sti * ST_TILES, min((sti + 1) * ST_TILES, TILES))
                    if ucnt[sti, wi, ti] > 0]
            slots = []
            tiles_in_blk = set()
            f = 0
            for (ti, n) in segs:
                if f > 0 and len(tiles_in_blk) >= 2 and ti not in tiles_in_blk:
                    slots.extend([-1] * (128 - f))
                    f = 0
                    tiles_in_blk = set()
                seg_start[(sti, wi, ti)] = total_slots + len(slots)
                rem = n
                while rem > 0:
                    take = min(128 - f, rem)
                    slots.extend([ti] * take)
                    f += take
                    rem -= take
                    tiles_in_blk.add(ti)
                    if f == 128:
                        f = 0
                        tiles_in_blk = set()
            if f > 0:
                slots.extend([-1] * (128 - f))
            n_slots = len(slots)
            nblk = n_slots // 128
            mm_list = []
            for b in range(nblk):
                blk = slots[b * 128:(b + 1) * 128]
                touched = []
                for q in blk:
                    if q >= 0 and q not in touched:
                        touched.append(q)
                for ti in touched:
                    mm_list.append((b, ti))
            st_runs.append({"w": wi, "n_slots": n_slots, "mm_list": mm_list,
                            "slot_base": total_slots, "mm_base": total_mms})
            slot_tile_all.extend(slots)
            total_slots += n_slots
            total_mms += len(mm_list)
        structure.append(st_runs)

    slot_tile_all = np.asarray(slot_tile_all, dtype=np.int64)

    # per-core: place edges into unified slots
    idx = np.zeros((NCORES, total_slots), dtype=np.int16)
    selcol = np.full((NCORES, total_slots), -1, dtype=np.int64)  # src col (s%128)
    for c in range(NCORES):
        s, d, w, t, sti = per_core[c]
        # edges sorted by (st, w, tile, s) -> contiguous per (st,w,tile)
        key = (sti * NWIN_G + w) * TILES + t
        change = np.empty(len(key), dtype=bool)
        if len(key):
            change[0] = True
            change[1:] = key[1:] != key[:-1]
        grp_start_idx = np.flatnonzero(change)
        grp_of_edge = np.cumsum(change) - 1
        offset_in_grp = np.arange(len(key)) - grp_start_idx[grp_of_edge]
        base = np.array([seg_start[(int(sti[i]), int(w[i]), int(t[i]))]
                         for i in grp_start_idx], dtype=np.int64)
        slot_pos = base[grp_of_edge] + offset_in_grp
        idx[c, slot_pos] = d.astype(np.int16)
        selcol[c, slot_pos] = s & 127

    # selector blob: [128 partitions(slot within block), total_mms*128] values 0/1
    sel = np.zeros((NCORES, 128, total_mms * 128), dtype=np.uint8)
    mm_i_global = 0
    for sti in range(NST):
        for run in structure[sti]:
            sb = run["slot_base"]
            for (b, ti) in run["mm_list"]:
                sl0 = sb + b * 128
                tile_match = slot_tile_all[sl0:sl0 + 128] == ti
                for c in range(NCORES):
                    cols = selcol[c, sl0:sl0 + 128]
                    jj = np.flatnonzero(tile_match & (cols >= 0))
                    sel[c, jj, mm_i_global * 128 + cols[jj]] = 1
                mm_i_global += 1
    assert mm_i_global == total_mms

    # wrapped idx arrays: [128, total_slots/16]; slot i -> [i%16 (+16g), i//16]
    assert total_slots % 128 == 0
    idx_w = np.zeros((NCORES, 16, total_slots // 16), dtype=np.int16)
    ar = np.arange(total_slots)
    idx_w[:, ar % 16, ar // 16] = idx
    idx_w = np.tile(idx_w, (1, 8, 1))

    return {"structure": structure, "total_slots": total_slots, "total_mms": total_mms,
            "idx": idx_w, "sel": sel}


def _prep(inputs):
    """All host-side preprocessing -> per-core input maps + static meta."""
    import ml_dtypes
    f16 = np.float16
    x = np.asarray(inputs["x"], np.float32)
    ei1 = np.asarray(inputs["edge_index_r1"])
    ei0 = np.asarray(inputs["edge_index_r0"])

    g1 = np.float32(_sigmoid(inputs["gate1"]))
    g0 = np.float32(_sigmoid(inputs["gate0"]))
    lns1 = np.asarray(inputs["lns1"], np.float32); lnb1 = np.asarray(inputs["lnb1"], np.float32)
    lns0 = np.asarray(inputs["lns0"], np.float32); lnb0 = np.asarray(inputs["lnb0"], np.float32)

    wl1 = np.asarray(inputs["wl1"], np.float32); bl1 = np.asarray(inputs["bl1"], np.float32)
    w01 = np.asarray(inputs["w01"], np.float32); b01 = np.asarray(inputs["b01"], np.float32)
    w11 = np.asarray(inputs["w11"], np.float32); b11 = np.asarray(inputs["b11"], np.float32)
    wl0 = np.asarray(inputs["wl0"], np.float32); bl0 = np.asarray(inputs["bl0"], np.float32)
    w00 = np.asarray(inputs["w00"], np.float32); b00 = np.asarray(inputs["b00"], np.float32)
    w10 = np.asarray(inputs["w10"], np.float32); b10 = np.asarray(inputs["b10"], np.float32)
    Wout = np.asarray(inputs["Wout"], np.float32); bout = np.asarray(inputs["bout"], np.float32)

    # Layer 1 (uses r1 edges, params *1); h = x0 = x
    rhsAB1 = np.concatenate([wl1.T, ((1 - g1) * w01 + g1 * w11).T], axis=1)  # [256,512]
    # bias row zero-padded to full acc width so the bias matmul closes the
    # whole PSUM accumulation group (hw half gets no bias)
    bias1 = np.concatenate([np.zeros(256, np.float32),
                            bl1 + (1 - g1) * b01 + g1 * b11])                 # [512]
    # Layer 2 (r0 edges, params *0); h = h1 = h1r*lns1 + lnb1, x0 = x
    wl0T_s = (lns1[:, None] * wl0.T)            # [256,256] for h1r path
    const1 = lnb1 @ wl0.T                        # [256] message const
    w00T_s = (1 - g0) * (lns1[:, None] * w00.T)
    rhsA2 = np.concatenate([wl0T_s, w00T_s], axis=1)  # [256,512]
    w10T_s = g0 * w10.T                          # [256,256] x path
    bias2 = bl0 + (1 - g0) * (b00 + lnb1 @ w00.T) + g0 * b10
    crow2 = np.concatenate([const1, bias2])      # [512]
    # Final: out = h2 @ WoutT + bout, h2 = h2r*lns0 + lnb0
    WoutT_s = lns0[:, None] * Wout.T             # [256,128]
    bout_s = bout + lnb0 @ Wout.T                # [128]

    # degree normalization (bincount over dst of the FULL edge list)
    inv1 = 1.0 / np.clip(np.bincount(ei1[1], minlength=N), 1.0, None).astype(np.float32)
    inv0 = 1.0 / np.clip(np.bincount(ei0[1], minlength=N), 1.0, None).astype(np.float32)

    # First N entries of each edge list are the arange self-edges (src==dst
    # by construction); they are handled on-device by one identity matmul per
    # tile instead of gather slots. Random self-edges (rare) stay in slots.
    lay1 = _build_layer(ei1[0][N:].astype(np.int64), ei1[1][N:].astype(np.int64))
    lay2 = _build_layer(ei0[0][N:].astype(np.int64), ei0[1][N:].astype(np.int64))

    fp8 = ml_dtypes.float8_e4m3
    in_maps = []
    for c in range(NCORES):
        lo = c * NPC
        xs = np.zeros((NPC_PAD, C), np.float32)
        xs[:NPC] = x[lo:lo + NPC]
        inv1c = np.zeros(NPC_PAD, np.float32); inv1c[:NPC] = inv1[lo:lo + NPC]
        inv0c = np.zeros(NPC_PAD, np.float32); inv0c[:NPC] = inv0[lo:lo + NPC]
        in_maps.append(dict(
            xT=np.ascontiguousarray(xs.T).astype(f16),
            rhsAB1=rhsAB1.astype(f16), brow1=bias1[None, :].astype(f16),
            rhsA2=rhsA2.astype(f16), rhsY2=w10T_s.astype(f16), crow2=crow2[None, :].astype(f16),
            rhsF=WoutT_s.astype(f16), browF=bout_s[None, :].astype(f16),
            invdeg1=np.ascontiguousarray(inv1c.reshape(TILES, P).T),
            invdeg2=np.ascontiguousarray(inv0c.reshape(TILES, P).T),
            idx1=lay1["idx"][c], idx2=lay2["idx"][c],
            sel1=lay1["sel"][c].astype(np.float32).astype(fp8),
            sel2=lay2["sel"][c].astype(np.float32).astype(fp8),
        ))
    return in_maps, lay1, lay2


# ---------------------------------------------------------------- device side
def _build_nc(lay1, lay2):
    import concourse.bass as bass
    import concourse.tile as tile
    from concourse import bacc, mybir
    from concourse.masks import make_identity

    f32, f16 = mybir.dt.float32, mybir.dt.float16
    f8, i16 = mybir.dt.float8e4, mybir.dt.int16
    AF = mybir.ActivationFunctionType
    OP = mybir.AluOpType

    nc = bacc.Bacc("TRN2", target_bir_lowering=False, debug=False, num_devices=NCORES)

    S1, S2 = lay1["total_slots"], lay2["total_slots"]
    M1, M2 = lay1["total_mms"], lay2["total_mms"]

    xT_in = nc.dram_tensor("xT", [C, NPC_PAD], f16, kind="ExternalInput").ap()
    rhsAB1_in = nc.dram_tensor("rhsAB1", [C, 512], f16, kind="ExternalInput").ap()
    brow1_in = nc.dram_tensor("brow1", [1, 512], f16, kind="ExternalInput").ap()
    rhsA2_in = nc.dram_tensor("rhsA2", [C, 512], f16, kind="ExternalInput").ap()
    rhsY2_in = nc.dram_tensor("rhsY2", [C, 256], f16, kind="ExternalInput").ap()
    crow2_in = nc.dram_tensor("crow2", [1, 512], f16, kind="ExternalInput").ap()
    rhsF_in = nc.dram_tensor("rhsF", [C, OUT], f16, kind="ExternalInput").ap()
    browF_in = nc.dram_tensor("browF", [1, OUT], f16, kind="ExternalInput").ap()
    invdeg1_in = nc.dram_tensor("invdeg1", [P, TILES], f32, kind="ExternalInput").ap()
    invdeg2_in = nc.dram_tensor("invdeg2", [P, TILES], f32, kind="ExternalInput").ap()
    idx1_in = nc.dram_tensor("idx1", [P, S1 // 16], i16, kind="ExternalInput").ap()
    idx2_in = nc.dram_tensor("idx2", [P, S2 // 16], i16, kind="ExternalInput").ap()
    sel1_in = nc.dram_tensor("sel1", [P, M1 * P], f8, kind="ExternalInput").ap()
    sel2_in = nc.dram_tensor("sel2", [P, M2 * P], f8, kind="ExternalInput").ap()

    out_dram = nc.dram_tensor("out", [NPC_PAD, OUT], f32, kind="ExternalOutput").ap()

    fm = f8 if FP8 else f16   # message/table dtype
    ag1 = nc.dram_tensor("ag1", [NPC, C], fm)
    ag2 = nc.dram_tensor("ag2", [NPC, C], fm)
    if NCHUNK == 1:
        t1 = nc.dram_tensor("table1", [N, C], fm, addr_space="Shared")
        t2 = nc.dram_tensor("table2", [N, C], fm, addr_space="Shared")
        tab1 = [ag1[:]] + [t1[w * WIN: w * WIN + min(32768, N - w * WIN), :]
                           for w in range(NWIN)]
        tab2 = [ag2[:]] + [t2[w * WIN: w * WIN + min(32768, N - w * WIN), :]
                           for w in range(NWIN)]
    else:
        t1 = [nc.dram_tensor(f"table1_{k}", [WIN_ROWS[k], C], fm, addr_space="Shared")
              for k in range(NWIN)]
        t2 = [nc.dram_tensor(f"table2_{k}", [WIN_ROWS[k], C], fm, addr_space="Shared")
              for k in range(NWIN)]
        tab1 = [ag1[:]] + [tb[:] for tb in t1]
        tab2 = [ag2[:]] + [tb[:] for tb in t2]
    agt1, agt2 = (ag1, t1), (ag2, t2)

    # max blocks in a single (st, window) run -> g-buffer slot size
    max_run_blk = 0
    max_mm_per_st = 0
    for lay in (lay1, lay2):
        for st_runs in lay["structure"]:
            nm = sum(len(r["mm_list"]) for r in st_runs)
            max_mm_per_st = max(max_mm_per_st, nm)
            for r in st_runs:
                max_run_blk = max(max_run_blk, r["n_slots"] // 128)

    with tile.TileContext(nc) as tc, ExitStack() as ctx:
        sb = ctx.enter_context(tc.tile_pool(name="sb", bufs=1))
        slab_pool = ctx.enter_context(tc.tile_pool(name="slab", bufs=2))
        lhs_pool = ctx.enter_context(tc.tile_pool(name="lhs", bufs=2))
        hwst = ctx.enter_context(tc.tile_pool(name="hwst", bufs=1))
        gpool = ctx.enter_context(tc.tile_pool(name="gst", bufs=3))
        spool = ctx.enter_context(tc.tile_pool(name="sel", bufs=2))
        epi = ctx.enter_context(tc.tile_pool(name="epi", bufs=2))
        hT_pool = ctx.enter_context(tc.tile_pool(name="hT", bufs=2))
        # PSUM: 8 banks total = cps 4 + abps 2 + trps 2
        cps = ctx.enter_context(tc.tile_pool(name="cps", bufs=4, space="PSUM"))
        abps = ctx.enter_context(tc.tile_pool(name="abps", bufs=2, space="PSUM"))
        trps = ctx.enter_context(tc.tile_pool(name="trps", bufs=2, space="PSUM"))

        # persistent staging
        z_sb = sb.tile([P, TILES * C], f16)      # dense term, then relu output r
        h_sb = sb.tile([P, TILES * C], f16)      # LN output (h1r, then h2r)
        s1_all = sb.tile([P, TILES], f32)
        s2_all = sb.tile([P, TILES], f32)
        mu_all = sb.tile([P, TILES], f32)
        rstd_all = sb.tile([P, TILES], f32)
        invdeg1_sb = sb.tile([P, TILES], f32)
        invdeg2_sb = sb.tile([P, TILES], f32)
        ident16 = sb.tile([P, P], f16)
        make_identity(nc, ident16[:])
        ones_col = sb.tile([1, P], f16)
        nc.vector.memset(ones_col[:], 1.0)
        nc.sync.dma_start(invdeg1_sb[:], invdeg1_in[:])
        nc.sync.dma_start(invdeg2_sb[:], invdeg2_in[:])

        # weight tiles (persistent)
        rhsAB1_sb = sb.tile([C // 2, 2, 512], f16)
        nc.sync.dma_start(rhsAB1_sb[:], rhsAB1_in[:].rearrange("(b k) n -> k b n", k=128))
        brow1_sb = sb.tile([1, 512], f16)
        nc.sync.dma_start(brow1_sb[:], brow1_in[:])
        rhsA2_sb = sb.tile([C // 2, 2, 512], f16)
        nc.sync.dma_start(rhsA2_sb[:], rhsA2_in[:].rearrange("(b k) n -> k b n", k=128))
        rhsY2_sb = sb.tile([C // 2, 2, 256], f16)
        nc.sync.dma_start(rhsY2_sb[:], rhsY2_in[:].rearrange("(b k) n -> k b n", k=128))
        crow2_sb = sb.tile([1, 512], f16)
        nc.sync.dma_start(crow2_sb[:], crow2_in[:])
        rhsF_sb = sb.tile([C // 2, 2, OUT], f16)
        nc.sync.dma_start(rhsF_sb[:], rhsF_in[:].rearrange("(b k) n -> k b n", k=128))
        browF_sb = sb.tile([1, OUT], f16)
        nc.sync.dma_start(browF_sb[:], browF_in[:])

        idx1_sb = sb.tile([P, S1 // 16], i16)
        nc.sync.dma_start(idx1_sb[:], idx1_in[:])
        idx2_sb = sb.tile([P, S2 // 16], i16)
        nc.sync.dma_start(idx2_sb[:], idx2_in[:])

        XC_T = 13   # tiles per xc load (half chunk; double-buffered)

        def load_xc(t0, nt, name):
            """One DMA for nt tiles' xT columns -> [P, 2, nt*P]."""
            xc = lhs_pool.tile([P, 2, XC_T * P], f16, tag="xc", name=name)
            nc.sync.dma_start(
                xc[:, :, 0:nt * P],
                xT_in[:, t0 * P:(t0 + nt) * P].rearrange("(b k) n -> k b n", k=128))
            return xc

        def dense_tile(layer, t, slab, tl, xc, xcl):
            """Compute [hw | z] for node-tile t; hw -> slab col tl, z -> z_sb."""
            acc = abps.tile([P, 512], f32, space="PSUM", tag="acc")
            lo, hi = xcl * P, (xcl + 1) * P
            if layer == 1:
                nc.tensor.matmul(acc[:], lhsT=xc[:, 0, lo:hi], rhs=rhsAB1_sb[:, 0, :], start=True, stop=False)
                nc.tensor.matmul(acc[:], lhsT=xc[:, 1, lo:hi], rhs=rhsAB1_sb[:, 1, :], start=False, stop=False)
                nc.tensor.matmul(acc[:], lhsT=ones_col[:], rhs=brow1_sb[:],
                                 start=False, stop=True, skip_group_check=True)
            else:
                # transpose h1r tile -> lhsT fp16
                hT = hT_pool.tile([P, 2, P], f16, tag="hT")
                for kk in range(2):
                    tp = trps.tile([P, P], f16, space="PSUM", tag="trp")
                    nc.tensor.transpose(tp[:], h_sb[:, t * C + kk * P: t * C + (kk + 1) * P], ident16[:])
                    nc.vector.tensor_copy(hT[:, kk, :], tp[:])
                nc.tensor.matmul(acc[:], lhsT=hT[:, 0, :], rhs=rhsA2_sb[:, 0, :], start=True, stop=False)
                nc.tensor.matmul(acc[:], lhsT=hT[:, 1, :], rhs=rhsA2_sb[:, 1, :], start=False, stop=False)
                nc.tensor.matmul(acc[:, 256:512], lhsT=xc[:, 0, lo:hi], rhs=rhsY2_sb[:, 0, :],
                                 start=False, stop=False, skip_group_check=True)
                nc.tensor.matmul(acc[:, 256:512], lhsT=xc[:, 1, lo:hi], rhs=rhsY2_sb[:, 1, :],
                                 start=False, stop=False, skip_group_check=True)
                nc.tensor.matmul(acc[:], lhsT=ones_col[:], rhs=crow2_sb[:],
                                 start=False, stop=True, skip_group_check=True)
            # hw half -> fp16 slab (scalar engine), z half -> z_sb (vector engine)
            nc.scalar.activation(slab[:, tl * C:(tl + 1) * C], acc[:, 0:256], AF.Copy)
            nc.vector.tensor_copy(z_sb[:, t * C:(t + 1) * C], acc[:, 256:512])

        def flush_chunk(k, slab, agt):
            """DMA chunk slab -> ag bounce; AllGather per chunk (NCHUNK=4) or
            once after the last chunk (NCHUNK=1)."""
            agd, tabs_t = agt
            rows = CHUNK_ROWS[k]
            full_t = rows // P           # full tiles in this chunk
            rem = rows - full_t * P
            r0 = CHUNK_BASE_R[k]
            nc.sync.dma_start(
                agd[r0:r0 + full_t * P, :].rearrange("(t p) c -> p t c", p=P),
                slab[:, 0:full_t * C].rearrange("p (t c) -> p t c", c=C))
            if rem > 0:
                nc.sync.dma_start(agd[r0 + full_t * P:r0 + rows, :],
                                  slab[:rem, full_t * C:(full_t + 1) * C])
            if NOAG:
                return
            if NCHUNK == 1:
                if k == NWIN - 1:
                    nc.gpsimd.collective_compute(
                        "AllGather", mybir.AluOpType.bypass,
                        replica_groups=[list(range(NCORES))],
                        ins=[agd[:].opt()], outs=[tabs_t[:].opt()])
            else:
                nc.gpsimd.collective_compute(
                    "AllGather", mybir.AluOpType.bypass,
                    replica_groups=[list(range(NCORES))],
                    ins=[agd[r0:r0 + rows, :].opt()], outs=[tabs_t[k][:].opt()])

        def gather_st(lay, sti, tabs, idx_sb, sel_in_ap, agd_h):
            """Gathers + selector matmuls for super-tile sti. Returns acc tiles."""
            structure = lay["structure"]
            st_runs = structure[sti]
            st_t0 = sti * ST_TILES
            st_ntiles = min(ST_TILES, TILES - st_t0)
            st_nmm = sum(len(r["mm_list"]) for r in st_runs)
            sel_sb = spool.tile([P, max_mm_per_st * P], f8, tag="s")
            mm_b0 = st_runs[0]["mm_base"]
            nc.sync.dma_start(sel_sb[:, 0:st_nmm * P],
                              sel_in_ap[:, mm_b0 * P:(mm_b0 + st_nmm) * P])
            accs = [cps.tile([P, 512], f32, space="PSUM", tag="agg", name=f"agg{_i}")
                    for _i in range((st_ntiles + 1) // 2)]
            # self-edge term: contiguous reload of this st's own hw rows; one
            # identity matmul per tile replaces 128 gather slots each.
            r0 = st_t0 * P
            rows = min(NPC, r0 + st_ntiles * P) - r0
            full_t = rows // P
            rem = rows - full_t * P
            hw_sb = hwst.tile([P, ST_TILES, C], f16, tag="hwst")
            nc.sync.dma_start(
                hw_sb[:, 0:full_t, :],
                agd_h[r0:r0 + full_t * P, :].rearrange("(t p) c -> p t c", p=P))
            if rem > 0:
                nc.sync.dma_start(hw_sb[0:rem, full_t, :],
                                  agd_h[r0 + full_t * P:r0 + rows, :])
            # mm bookkeeping: global order across runs; first mm per BANK sets
            # start=True (clears whole bank), last mm per TILE sets stop=True.
            # run index -1 = the per-tile identity matmuls (emitted first).
            mm_seq = [(-1, tl, st_t0 + tl) for tl in range(st_ntiles)]
            for ri, run in enumerate(st_runs):
                for (b, ti) in run["mm_list"]:
                    mm_seq.append((ri, b, ti))
            last = {}
            for i, (_, _, ti) in enumerate(mm_seq):
                last[ti] = i
            bank_started = set()
            mm_i = 0
            for (_, tl, ti) in mm_seq[:st_ntiles]:
                kr = P if tl < full_t else rem
                bank = tl // 2
                reg = accs[bank][:, (tl % 2) * 256:(tl % 2) * 256 + 256]
                nc.tensor.matmul(
                    reg, lhsT=ident16[0:kr, :], rhs=hw_sb[0:kr, tl, :],
                    start=(bank not in bank_started), stop=(last[ti] == mm_i),
                    skip_group_check=True)
                bank_started.add(bank)
                mm_i += 1
            for ri, run in enumerate(st_runs):
                ns = run["n_slots"]
                if ns == 0:
                    continue
                w = run["w"]
                sb0 = run["slot_base"]
                run_mms = []
                for (ri2, b, ti) in mm_seq[mm_i:]:
                    if ri2 != ri:
                        break
                    run_mms.append((b, ti))
                # Per-1024-row chunk: small rotating g tiles (4 deep) so the
                # Pool->SDMA->PE chain pipelines at chunk granularity and mms
                # fire as soon as their chunk lands. SWDGE descriptor-ring
                # carveout holds 1024 descriptors; larger gathers wedge.
                GMAX = 1024
                pos = 0
                for g0 in range(0, ns, GMAX):
                    gn = min(GMAX, ns - g0)
                    gb = gn // 128
                    b0 = g0 // 128
                    g_t = gpool.tile([P, (GMAX // P) * C], fm, tag="g")
                    if not NOGATHER:
                        nc.gpsimd.dma_gather(
                            out_ap=g_t[:, 0:gb * C].rearrange("p (b c) -> p b c", c=C),
                            in_ap=tabs[w],
                            idxs_ap=idx_sb[:, (sb0 + g0) // 16:(sb0 + g0 + gn) // 16],
                            num_idxs=gn, num_idxs_reg=gn, elem_size=C,
                        )
                    while pos < len(run_mms) and run_mms[pos][0] < b0 + gb:
                        b, ti = run_mms[pos]
                        tl = ti - st_t0
                        bank = tl // 2
                        reg = accs[bank][:, (tl % 2) * 256:(tl % 2) * 256 + 256]
                        if not NOSEL:
                            si = mm_i - st_ntiles   # sel blob holds slot mms only
                            nc.tensor.matmul(
                                reg, lhsT=sel_sb[:, si * P:(si + 1) * P],
                                rhs=g_t[:, (b - b0) * C:(b - b0 + 1) * C],
                                start=(bank not in bank_started), stop=(last[ti] == mm_i),
                                skip_group_check=True,
                            )
                            bank_started.add(bank)
                        mm_i += 1
                        pos += 1
                assert pos == len(run_mms)
            assert mm_i == len(mm_seq)
            return accs

        def epilogue_st(sti, accs, invdeg_sb):
            """relu(agg*invdeg + z) + LN stats + normalize for super-tile sti."""
            st_t0 = sti * ST_TILES
            st_ntiles = min(ST_TILES, TILES - st_t0)
            for tl in range(st_ntiles):
                t = st_t0 + tl
                reg = accs[tl // 2][:, (tl % 2) * 256:(tl % 2) * 256 + 256]
                tmp = epi.tile([P, C], f32, tag="etmp")
                nc.vector.scalar_tensor_tensor(
                    out=tmp[:], in0=reg, scalar=invdeg_sb[:, t:t + 1],
                    in1=z_sb[:, t * C:(t + 1) * C], op0=OP.mult, op1=OP.add)
                nc.scalar.activation(z_sb[:, t * C:(t + 1) * C], tmp[:], AF.Relu,
                                     accum_out=s1_all[:, t:t + 1])
                sq = epi.tile([P, C], f16, tag="esq")
                nc.scalar.activation(sq[:], z_sb[:, t * C:(t + 1) * C], AF.Square,
                                     accum_out=s2_all[:, t:t + 1])
            # per-st stats: mu = s1/C ; var = s2/C - mu^2 ; rstd = 1/sqrt(var+eps)
            a, b = st_t0, st_t0 + st_ntiles
            nc.vector.tensor_scalar(out=mu_all[:, a:b], in0=s1_all[:, a:b],
                                    scalar1=1.0 / C, scalar2=None, op0=OP.mult)
            var = epi.tile([P, ST_TILES], f32, tag="evar")
            nc.vector.tensor_tensor(out=var[:, 0:b - a], in0=mu_all[:, a:b],
                                    in1=mu_all[:, a:b], op=OP.mult)
            nc.vector.scalar_tensor_tensor(out=var[:, 0:b - a], in0=s2_all[:, a:b],
                                           scalar=1.0 / C, in1=var[:, 0:b - a],
                                           op0=OP.mult, op1=OP.subtract)
            nc.vector.tensor_scalar(out=var[:, 0:b - a], in0=var[:, 0:b - a],
                                    scalar1=float(LN_EPS), scalar2=None, op0=OP.add)
            std = epi.tile([P, ST_TILES], f32, tag="estd")
            nc.scalar.activation(std[:, 0:b - a], var[:, 0:b - a], AF.Sqrt)
            nc.vector.reciprocal(rstd_all[:, a:b], std[:, 0:b - a])
            # normalize -> h_sb fp16
            for tl in range(st_ntiles):
                t = st_t0 + tl
                nc.vector.tensor_scalar(
                    out=h_sb[:, t * C:(t + 1) * C], in0=z_sb[:, t * C:(t + 1) * C],
                    scalar1=mu_all[:, t:t + 1], scalar2=rstd_all[:, t:t + 1],
                    op0=OP.subtract, op1=OP.mult)

        def final_tiles(t0, t1):
            """Project h2 tiles [t0,t1) -> out (DMA straight from PSUM)."""
            for t in range(t0, t1):
                hT = hT_pool.tile([P, 2, P], f16, tag="hT", name="fhT")
                for kk in range(2):
                    tp = trps.tile([P, P], f16, space="PSUM", tag="trp", name="ftp")
                    nc.tensor.transpose(tp[:], h_sb[:, t * C + kk * P: t * C + (kk + 1) * P], ident16[:])
                    nc.vector.tensor_copy(hT[:, kk, :], tp[:])
                acc = abps.tile([P, OUT], f32, space="PSUM", tag="acc", name="facc")
                nc.tensor.matmul(acc[:], lhsT=hT[:, 0, :], rhs=rhsF_sb[:, 0, :], start=True, stop=False)
                nc.tensor.matmul(acc[:], lhsT=hT[:, 1, :], rhs=rhsF_sb[:, 1, :], start=False, stop=False)
                nc.tensor.matmul(acc[:], lhsT=ones_col[:], rhs=browF_sb[:],
                                 start=False, stop=True, skip_group_check=True)
                o_sb = epi.tile([P, OUT], f32, tag="fo")
                nc.vector.tensor_copy(o_sb[:], acc[:])
                nc.sync.dma_start(out_dram[t * P:(t + 1) * P, :], o_sb[:])

        # ---------------- layer 1 dense, chunked AllGather
        for _rep in range(REPEAT):
          for k in range(NWIN):
            slab = slab_pool.tile([P, CHUNK_TILES[k] * C], fm, tag="slab")
            xc = None
            for tl in range(CHUNK_TILES[k]):
                if tl % XC_T == 0:
                    nt = min(XC_T, CHUNK_TILES[k] - tl)
                    xc = load_xc(CHUNK_T0[k] + tl, nt, f"xc1_{k}_{tl}")
                dense_tile(1, CHUNK_T0[k] + tl, slab, tl, xc, tl % XC_T)
            flush_chunk(k, slab, agt1)

          def dense2_range(lo, hi, state):
              """Dense-2 for tiles [lo,hi); fires AG2 chunks as they complete."""
              slab2 = state.get("slab2")
              xc2 = state.get("xc2")
              for t in range(lo, hi):
                  k = min(t // 25, 3)
                  tl = t - CHUNK_T0[k]
                  if slab2 is None:
                      slab2 = slab_pool.tile([P, CHUNK_TILES[k] * C], fm, tag="slab",
                                             name=f"slab2_{k}")
                  if tl % XC_T == 0:
                      nt = min(XC_T, CHUNK_TILES[k] - tl)
                      xc2 = load_xc(t, nt, f"xc2_{k}_{tl}")
                  dense_tile(2, t, slab2, tl, xc2, tl % XC_T)
                  if t == CHUNK_T0[k] + CHUNK_TILES[k] - 1:
                      flush_chunk(k, slab2, agt2)
                      slab2 = None
              state["slab2"] = slab2
              state["xc2"] = xc2

          if INTERLEAVE:
              # ---------------- gather1 with interleaved dense2 + chunked AG2
              d2state = {}
              dense2_done = 0
              for s in range(NST):
                  accs = gather_st(lay1, s, tab1, idx1_sb, sel1_in, ag1)
                  epilogue_st(s, accs, invdeg1_sb)
                  hi = min((s + 1) * ST_TILES, TILES)
                  dense2_range(dense2_done, hi, d2state)
                  dense2_done = hi
              # ---------------- gather2 with interleaved final projection
              for s in range(NST):
                  accs = gather_st(lay2, s, tab2, idx2_sb, sel2_in, ag2)
                  epilogue_st(s, accs, invdeg2_sb)
                  final_tiles(s * ST_TILES, min((s + 1) * ST_TILES, TILES))
          else:
              for s in range(NST):
                  accs = gather_st(lay1, s, tab1, idx1_sb, sel1_in, ag1)
                  epilogue_st(s, accs, invdeg1_sb)
              dense2_range(0, TILES, {})
              for s in range(NST):
                  accs = gather_st(lay2, s, tab2, idx2_sb, sel2_in, ag2)
                  epilogue_st(s, accs, invdeg2_sb)
              final_tiles(0, TILES)

    nc.compile()
    return nc


# ---------------------------------------------------------------- entry point
def kernel(**inputs):
    from concourse.bass_utils import run_bass_kernel_spmd

    in_maps, lay1, lay2 = _prep(inputs)
    key = "nc"
    if key not in _COMPILED:
        _COMPILED[key] = _build_nc(lay1, lay2)
    nc = _COMPILED[key]
    res = run_bass_kernel_spmd(nc, in_maps, core_ids=list(range(NCORES)))
    _COMPILED["last_res"] = res
    out = np.concatenate([res.results[c]["out"][:NPC] for c in range(NCORES)], axis=0)
    return out.astype(np.float32)



# revision 23
# speedup vs baseline: 1.0527x; 1.0527x over previous
"""MetaPathGNN forward on 8 Trainium2 NeuronCores (Bass/Tile).

Strategy (self-contained; shapes hardcoded for N=100000, C=256, OUT=128, E=400000):
  - Nodes sharded 12500/core. Per layer: each core computes hw = h @ wlT for its
    nodes (fp16), staged in an SBUF slab per chunk of 25 node-tiles, then
    AllGathered chunk-by-chunk (4 chunks) so the collective streams behind
    compute. Gather windows == chunk outputs (<=25600 rows, int16-addressable).
  - Edges assigned to cores by src owner; host sorts each core's edges by
    (super-tile(src), window(dst), tile(src)) and pads so the slot layout
    is identical across cores (single SPMD NEFF). The guaranteed arange
    self-edge of every node is handled by one identity matmul per tile fed
    from a contiguous reload of the core's own hw rows (removes 12.5k random
    gather slots per layer); random self-edges stay in slots.
  - Messages gathered with GpSimd dma_gather (512B fp16 rows, int16 indices
    against per-chunk tables). xT dense lhs loaded per 25-tile chunk in one
    1.6MB DMA instead of 98 64KB DMAs.
  - Segment-sum = fp8(0/1 selector) x fp16(messages) matmuls accumulated in
    PSUM, one [128,512] bank per pair of 128-node tiles.
  - Epilogue fuses deg-normalize + dense term, relu + LN stats, normalize;
    LN stats computed per super-tile so downstream work pipelines.
  - Layer-2 dense matmuls + its chunked AllGather are interleaved into
    layer-1's gather loop; the final projection is interleaved into layer-2's
    gather loop, with outputs DMAed straight from PSUM.
"""
import numpy as np
from contextlib import ExitStack

N = 100000
C = 256
OUT = 128
NCORES = 8
NPC = N // NCORES          # 12500 nodes per core
P = 128
TILES = (NPC + P - 1) // P  # 98
NPC_PAD = TILES * P         # 12544
ST_TILES = 8                # node-tiles per super-tile
NST = (TILES + ST_TILES - 1) // ST_TILES  # 13
LN_EPS = 1e-5

# AllGather chunking: 4 chunks of node-tiles per core; window w == chunk w.
CHUNK_TILES = [25, 25, 25, 23]
CHUNK_T0 = [0, 25, 50, 75]
CHUNK_ROWS = [3200, 3200, 3200, 2900]   # real (unpadded) rows per core
CHUNK_BASE_R = [0, 3200, 6400, 9600]
NWIN = 4
NWIN_G = 5  # gather windows: 0 = local (own ag bounce), 1..4 = table windows
WIN_ROWS = [r * NCORES for r in CHUNK_ROWS]  # [25600,25600,25600,23200]

import os as _os
INTERLEAVE = _os.environ.get("K_INTERLEAVE", "1") == "1"
# NCHUNK=4: per-chunk tables + 4 AllGathers/layer. NCHUNK=1: one table +
# one AllGather/layer with overlapping 32768-row windows (baseline scheme).
NCHUNK = int(_os.environ.get("K_NCHUNK", "4"))
# FP8 message tables: hw staged/gathered/exchanged as float8e4 instead of
# fp16 — halves AllGather wire bytes and gather DMA traffic.
FP8 = _os.environ.get("K_FP8", "0") == "1"  # fails 2e-2 gate (3.7e-2): deg-1
# nodes take a full single-row fp8 quantization hit; keep fp16 tables.
REPEAT = int(_os.environ.get("K_REPEAT", "1"))  # bench: repeat pipeline in-NEFF
NOAG = _os.environ.get("K_NOAG", "0") == "1"     # bench: skip collectives (wrong results)
NOGATHER = _os.environ.get("K_NOGATHER", "0") == "1"  # bench: skip dma_gathers
NOSEL = _os.environ.get("K_NOSEL", "0") == "1"   # bench: skip selector matmuls
WIN = 25000  # window stride in single-table mode

_COMPILED = {}


# ---------------------------------------------------------------- host side
def _sigmoid(x):
    return 1.0 / (1.0 + np.exp(-np.float64(x)))


def _build_layer(src, dst):
    """Vectorized layout builder. Returns dict with:
       structure: per st -> list of (w, n_slots, mm_list(block,tile), slot_base, mm_base)
       idx:  [NCORES, S] int16 window-local gather indices
       sel:  [NCORES, 128, NMM*128] 0/1 fp8 selector blob
    """
    rows_w = np.asarray(CHUNK_ROWS, dtype=np.int64)
    base_w = np.asarray(CHUNK_BASE_R, dtype=np.int64)
    per_core = []
    for c in range(NCORES):
        lo = c * NPC
        m = (src >= lo) & (src < lo + NPC)
        s = (src[m] - lo).astype(np.int64)
        dg = dst[m].astype(np.int64)
        # window 0 = LOCAL: dst owned by this core (incl. all self-edges),
        # gathered from the core's own ag bounce buffer -- uniform across
        # cores and independent of the collective. windows 1..4 = table.
        is_local = (dg >= lo) & (dg < lo + NPC)
        if NCHUNK == 1:
            # single node-major table; overlapping windows of stride WIN
            wt = np.minimum(dg // WIN, 3)
            dloc_t = dg - wt * WIN
        else:
            dc = dg // NPC
            dr = dg - dc * NPC
            wt = np.minimum(dr // 3200, 3)
            dloc_t = dc * rows_w[wt] + (dr - base_w[wt])
        w = np.where(is_local, 0, wt + 1)
        dloc = np.where(is_local, dg - lo, dloc_t)
        t = s >> 7
        sti = t // ST_TILES
        order = np.lexsort((s, t, w, sti))
        per_core.append((s[order], dloc[order], w[order], t[order], sti[order]))

    cnt = np.zeros((NCORES, NST, NWIN_G, TILES), dtype=np.int64)
    for c in range(NCORES):
        s, d, w, t, sti = per_core[c]
        np.add.at(cnt[c], (sti, w, t), 1)
    ucnt = cnt.max(axis=0)

    structure = []
    total_slots = 0
    total_mms = 0
    slot_tile_all = []
    seg_start = {}  # (st,w,tile) -> unified slot start position
    for sti in range(NST):
        st_runs = []
        for wi in range(NWIN_G):
            segs = [(ti, int(ucnt[sti, wi, ti]))
                    for ti in range(sti * ST_TILES, min((sti + 1) * ST_TILES, TILES))
                    if ucnt[sti, wi, ti] > 0]
            slots = []
            tiles_in_blk = set()
            f = 0
            for (ti, n) in segs:
                if f > 0 and len(tiles_in_blk) >= 2 and ti not in tiles_in_blk:
                    slots.extend([-1] * (128 - f))
                    f = 0
                    tiles_in_blk = set()
                seg_start[(sti, wi, ti)] = total_slots + len(slots)
                rem = n
                while rem > 0:
                    take = min(128 - f, rem)
                    slots.extend([ti] * take)
                    f += take
                    rem -= take
                    tiles_in_blk.add(ti)
                    if f == 128:
                        f = 0
                        tiles_in_blk = set()
            if f > 0:
                slots.extend([-1] * (128 - f))
            n_slots = len(slots)
            nblk = n_slots // 128
            mm_list = []
            for b in range(nblk):
                blk = slots[b * 128:(b + 1) * 128]
                touched = []
                for q in blk:
                    if q >= 0 and q not in touched:
                        touched.append(q)
                for ti in touched:
                    mm_list.append((b, ti))
            st_runs.append({"w": wi, "n_slots": n_slots, "mm_list": mm_list,
                            "slot_base": total_slots, "mm_base": total_mms})
            slot_tile_all.extend(slots)
            total_slots += n_slots
            total_mms += len(mm_list)
        structure.append(st_runs)

    slot_tile_all = np.asarray(slot_tile_all, dtype=np.int64)

    # per-core: place edges into unified slots
    idx = np.zeros((NCORES, total_slots), dtype=np.int16)
    selcol = np.full((NCORES, total_slots), -1, dtype=np.int64)  # src col (s%128)
    for c in range(NCORES):
        s, d, w, t, sti = per_core[c]
        # edges sorted by (st, w, tile, s) -> contiguous per (st,w,tile)
        key = (sti * NWIN_G + w) * TILES + t
        change = np.empty(len(key), dtype=bool)
        if len(key):
            change[0] = True
            change[1:] = key[1:] != key[:-1]
        grp_start_idx = np.flatnonzero(change)
        grp_of_edge = np.cumsum(change) - 1
        offset_in_grp = np.arange(len(key)) - grp_start_idx[grp_of_edge]
        base = np.array([seg_start[(int(sti[i]), int(w[i]), int(t[i]))]
                         for i in grp_start_idx], dtype=np.int64)
        slot_pos = base[grp_of_edge] + offset_in_grp
        idx[c, slot_pos] = d.astype(np.int16)
        selcol[c, slot_pos] = s & 127

    # selector blob: [128 partitions(slot within block), total_mms*128] values 0/1
    sel = np.zeros((NCORES, 128, total_mms * 128), dtype=np.uint8)
    mm_i_global = 0
    for sti in range(NST):
        for run in structure[sti]:
            sb = run["slot_base"]
            for (b, ti) in run["mm_list"]:
                sl0 = sb + b * 128
                tile_match = slot_tile_all[sl0:sl0 + 128] == ti
                for c in range(NCORES):
                    cols = selcol[c, sl0:sl0 + 128]
                    jj = np.flatnonzero(tile_match & (cols >= 0))
                    sel[c, jj, mm_i_global * 128 + cols[jj]] = 1
                mm_i_global += 1
    assert mm_i_global == total_mms

    # wrapped idx arrays: [128, total_slots/16]; slot i -> [i%16 (+16g), i//16]
    assert total_slots % 128 == 0
    idx_w = np.zeros((NCORES, 16, total_slots // 16), dtype=np.int16)
    ar = np.arange(total_slots)
    idx_w[:, ar % 16, ar // 16] = idx
    idx_w = np.tile(idx_w, (1, 8, 1))

    return {"structure": structure, "total_slots": total_slots, "total_mms": total_mms,
            "idx": idx_w, "sel": sel}


def _prep(inputs):
    """All host-side preprocessing -> per-core input maps + static meta."""
    import ml_dtypes
    f16 = np.float16
    x = np.asarray(inputs["x"], np.float32)
    ei1 = np.asarray(inputs["edge_index_r1"])
    ei0 = np.asarray(inputs["edge_index_r0"])

    g1 = np.float32(_sigmoid(inputs["gate1"]))
    g0 = np.float32(_sigmoid(inputs["gate0"]))
    lns1 = np.asarray(inputs["lns1"], np.float32); lnb1 = np.asarray(inputs["lnb1"], np.float32)
    lns0 = np.asarray(inputs["lns0"], np.float32); lnb0 = np.asarray(inputs["lnb0"], np.float32)

    wl1 = np.asarray(inputs["wl1"], np.float32); bl1 = np.asarray(inputs["bl1"], np.float32)
    w01 = np.asarray(inputs["w01"], np.float32); b01 = np.asarray(inputs["b01"], np.float32)
    w11 = np.asarray(inputs["w11"], np.float32); b11 = np.asarray(inputs["b11"], np.float32)
    wl0 = np.asarray(inputs["wl0"], np.float32); bl0 = np.asarray(inputs["bl0"], np.float32)
    w00 = np.asarray(inputs["w00"], np.float32); b00 = np.asarray(inputs["b00"], np.float32)
    w10 = np.asarray(inputs["w10"], np.float32); b10 = np.asarray(inputs["b10"], np.float32)
    Wout = np.asarray(inputs["Wout"], np.float32); bout = np.asarray(inputs["bout"], np.float32)

    # Layer 1 (uses r1 edges, params *1); h = x0 = x
    rhsAB1 = np.concatenate([wl1.T, ((1 - g1) * w01 + g1 * w11).T], axis=1)  # [256,512]
    # bias row zero-padded to full acc width so the bias matmul closes the
    # whole PSUM accumulation group (hw half gets no bias)
    bias1 = np.concatenate([np.zeros(256, np.float32),
                            bl1 + (1 - g1) * b01 + g1 * b11])                 # [512]
    # Layer 2 (r0 edges, params *0); h = h1 = h1r*lns1 + lnb1, x0 = x
    wl0T_s = (lns1[:, None] * wl0.T)            # [256,256] for h1r path
    const1 = lnb1 @ wl0.T                        # [256] message const
    w00T_s = (1 - g0) * (lns1[:, None] * w00.T)
    rhsA2 = np.concatenate([wl0T_s, w00T_s], axis=1)  # [256,512]
    w10T_s = g0 * w10.T                          # [256,256] x path
    bias2 = bl0 + (1 - g0) * (b00 + lnb1 @ w00.T) + g0 * b10
    crow2 = np.concatenate([const1, bias2])      # [512]
    # Final: out = h2 @ WoutT + bout, h2 = h2r*lns0 + lnb0
    WoutT_s = lns0[:, None] * Wout.T             # [256,128]
    bout_s = bout + lnb0 @ Wout.T                # [128]

    # degree normalization (bincount over dst of the FULL edge list)
    inv1 = 1.0 / np.clip(np.bincount(ei1[1], minlength=N), 1.0, None).astype(np.float32)
    inv0 = 1.0 / np.clip(np.bincount(ei0[1], minlength=N), 1.0, None).astype(np.float32)

    # First N entries of each edge list are the arange self-edges (src==dst
    # by construction); they are handled on-device by one identity matmul per
    # tile instead of gather slots. Random self-edges (rare) stay in slots.
    lay1 = _build_layer(ei1[0][N:].astype(np.int64), ei1[1][N:].astype(np.int64))
    lay2 = _build_layer(ei0[0][N:].astype(np.int64), ei0[1][N:].astype(np.int64))

    fp8 = ml_dtypes.float8_e4m3
    in_maps = []
    for c in range(NCORES):
        lo = c * NPC
        xs = np.zeros((NPC_PAD, C), np.float32)
        xs[:NPC] = x[lo:lo + NPC]
        inv1c = np.zeros(NPC_PAD, np.float32); inv1c[:NPC] = inv1[lo:lo + NPC]
        inv0c = np.zeros(NPC_PAD, np.float32); inv0c[:NPC] = inv0[lo:lo + NPC]
        in_maps.append(dict(
            xT=np.ascontiguousarray(xs.T).astype(f16),
            rhsAB1=rhsAB1.astype(f16), brow1=bias1[None, :].astype(f16),
            rhsA2=rhsA2.astype(f16), rhsY2=w10T_s.astype(f16), crow2=crow2[None, :].astype(f16),
            rhsF=WoutT_s.astype(f16), browF=bout_s[None, :].astype(f16),
            invdeg1=np.ascontiguousarray(inv1c.reshape(TILES, P).T),
            invdeg2=np.ascontiguousarray(inv0c.reshape(TILES, P).T),
            idx1=lay1["idx"][c], idx2=lay2["idx"][c],
            sel1=lay1["sel"][c].astype(np.float32).astype(fp8),
            sel2=lay2["sel"][c].astype(np.float32).astype(fp8),
        ))
    return in_maps, lay1, lay2


# ---------------------------------------------------------------- device side
def _build_nc(lay1, lay2):
    import concourse.bass as bass
    import concourse.tile as tile
    from concourse import bacc, mybir
    from concourse.masks import make_identity

    f32, f16 = mybir.dt.float32, mybir.dt.float16
    f8, i16 = mybir.dt.float8e4, mybir.dt.int16
    AF = mybir.ActivationFunctionType
    OP = mybir.AluOpType

    nc = bacc.Bacc("TRN2", target_bir_lowering=False, debug=False, num_devices=NCORES)

    S1, S2 = lay1["total_slots"], lay2["total_slots"]
    M1, M2 = lay1["total_mms"], lay2["total_mms"]

    xT_in = nc.dram_tensor("xT", [C, NPC_PAD], f16, kind="ExternalInput").ap()
    rhsAB1_in = nc.dram_tensor("rhsAB1", [C, 512], f16, kind="ExternalInput").ap()
    brow1_in = nc.dram_tensor("brow1", [1, 512], f16, kind="ExternalInput").ap()
    rhsA2_in = nc.dram_tensor("rhsA2", [C, 512], f16, kind="ExternalInput").ap()
    rhsY2_in = nc.dram_tensor("rhsY2", [C, 256], f16, kind="ExternalInput").ap()
    crow2_in = nc.dram_tensor("crow2", [1, 512], f16, kind="ExternalInput").ap()
    rhsF_in = nc.dram_tensor("rhsF", [C, OUT], f16, kind="ExternalInput").ap()
    browF_in = nc.dram_tensor("browF", [1, OUT], f16, kind="ExternalInput").ap()
    invdeg1_in = nc.dram_tensor("invdeg1", [P, TILES], f32, kind="ExternalInput").ap()
    invdeg2_in = nc.dram_tensor("invdeg2", [P, TILES], f32, kind="ExternalInput").ap()
    idx1_in = nc.dram_tensor("idx1", [P, S1 // 16], i16, kind="ExternalInput").ap()
    idx2_in = nc.dram_tensor("idx2", [P, S2 // 16], i16, kind="ExternalInput").ap()
    sel1_in = nc.dram_tensor("sel1", [P, M1 * P], f8, kind="ExternalInput").ap()
    sel2_in = nc.dram_tensor("sel2", [P, M2 * P], f8, kind="ExternalInput").ap()

    out_dram = nc.dram_tensor("out", [NPC_PAD, OUT], f32, kind="ExternalOutput").ap()

    fm = f8 if FP8 else f16   # message/table dtype
    ag1 = nc.dram_tensor("ag1", [NPC, C], fm)
    ag2 = nc.dram_tensor("ag2", [NPC, C], fm)
    if NCHUNK == 1:
        t1 = nc.dram_tensor("table1", [N, C], fm, addr_space="Shared")
        t2 = nc.dram_tensor("table2", [N, C], fm, addr_space="Shared")
        tab1 = [ag1[:]] + [t1[w * WIN: w * WIN + min(32768, N - w * WIN), :]
                           for w in range(NWIN)]
        tab2 = [ag2[:]] + [t2[w * WIN: w * WIN + min(32768, N - w * WIN), :]
                           for w in range(NWIN)]
    else:
        t1 = [nc.dram_tensor(f"table1_{k}", [WIN_ROWS[k], C], fm, addr_space="Shared")
              for k in range(NWIN)]
        t2 = [nc.dram_tensor(f"table2_{k}", [WIN_ROWS[k], C], fm, addr_space="Shared")
              for k in range(NWIN)]
        tab1 = [ag1[:]] + [tb[:] for tb in t1]
        tab2 = [ag2[:]] + [tb[:] for tb in t2]
    agt1, agt2 = (ag1, t1), (ag2, t2)

    # max blocks in a single (st, window) run -> g-buffer slot size
    max_run_blk = 0
    max_mm_per_st = 0
    for lay in (lay1, lay2):
        for st_runs in lay["structure"]:
            nm = sum(len(r["mm_list"]) for r in st_runs)
            max_mm_per_st = max(max_mm_per_st, nm)
            for r in st_runs:
                max_run_blk = max(max_run_blk, r["n_slots"] // 128)

    with tile.TileContext(nc) as tc, ExitStack() as ctx:
        sb = ctx.enter_context(tc.tile_pool(name="sb", bufs=1))
        slab_pool = ctx.enter_context(tc.tile_pool(name="slab", bufs=2))
        lhs_pool = ctx.enter_context(tc.tile_pool(name="lhs", bufs=2))
        hwst = ctx.enter_context(tc.tile_pool(name="hwst", bufs=2))
        gpool = ctx.enter_context(tc.tile_pool(name="gst", bufs=4))
        spool = ctx.enter_context(tc.tile_pool(name="sel", bufs=2))
        epi = ctx.enter_context(tc.tile_pool(name="epi", bufs=2))
        hT_pool = ctx.enter_context(tc.tile_pool(name="hT", bufs=2))
        # PSUM: 8 banks total = cps 4 + abps 2 + trps 2
        cps = ctx.enter_context(tc.tile_pool(name="cps", bufs=4, space="PSUM"))
        abps = ctx.enter_context(tc.tile_pool(name="abps", bufs=2, space="PSUM"))
        trps = ctx.enter_context(tc.tile_pool(name="trps", bufs=2, space="PSUM"))

        # persistent staging
        z_sb = sb.tile([P, TILES * C], f16)      # dense term, then relu output r
        h_sb = sb.tile([P, TILES * C], f16)      # LN output (h1r, then h2r)
        s1_all = sb.tile([P, TILES], f32)
        s2_all = sb.tile([P, TILES], f32)
        mu_all = sb.tile([P, TILES], f32)
        rstd_all = sb.tile([P, TILES], f32)
        invdeg1_sb = sb.tile([P, TILES], f32)
        invdeg2_sb = sb.tile([P, TILES], f32)
        ident16 = sb.tile([P, P], f16)
        make_identity(nc, ident16[:])
        ones_col = sb.tile([1, P], f16)
        nc.vector.memset(ones_col[:], 1.0)
        nc.sync.dma_start(invdeg1_sb[:], invdeg1_in[:])
        nc.sync.dma_start(invdeg2_sb[:], invdeg2_in[:])

        # weight tiles (persistent)
        rhsAB1_sb = sb.tile([C // 2, 2, 512], f16)
        nc.sync.dma_start(rhsAB1_sb[:], rhsAB1_in[:].rearrange("(b k) n -> k b n", k=128))
        brow1_sb = sb.tile([1, 512], f16)
        nc.sync.dma_start(brow1_sb[:], brow1_in[:])
        rhsA2_sb = sb.tile([C // 2, 2, 512], f16)
        nc.sync.dma_start(rhsA2_sb[:], rhsA2_in[:].rearrange("(b k) n -> k b n", k=128))
        rhsY2_sb = sb.tile([C // 2, 2, 256], f16)
        nc.sync.dma_start(rhsY2_sb[:], rhsY2_in[:].rearrange("(b k) n -> k b n", k=128))
        crow2_sb = sb.tile([1, 512], f16)
        nc.sync.dma_start(crow2_sb[:], crow2_in[:])
        rhsF_sb = sb.tile([C // 2, 2, OUT], f16)
        nc.sync.dma_start(rhsF_sb[:], rhsF_in[:].rearrange("(b k) n -> k b n", k=128))
        browF_sb = sb.tile([1, OUT], f16)
        nc.sync.dma_start(browF_sb[:], browF_in[:])

        idx1_sb = sb.tile([P, S1 // 16], i16)
        nc.sync.dma_start(idx1_sb[:], idx1_in[:])
        idx2_sb = sb.tile([P, S2 // 16], i16)
        nc.sync.dma_start(idx2_sb[:], idx2_in[:])

        XC_T = 13   # tiles per xc load (half chunk; double-buffered)

        def load_xc(t0, nt, name):
            """One DMA for nt tiles' xT columns -> [P, 2, nt*P]."""
            xc = lhs_pool.tile([P, 2, XC_T * P], f16, tag="xc", name=name)
            nc.sync.dma_start(
                xc[:, :, 0:nt * P],
                xT_in[:, t0 * P:(t0 + nt) * P].rearrange("(b k) n -> k b n", k=128))
            return xc

        def dense_tile(layer, t, slab, tl, xc, xcl):
            """Compute [hw | z] for node-tile t; hw -> slab col tl, z -> z_sb."""
            acc = abps.tile([P, 512], f32, space="PSUM", tag="acc")
            lo, hi = xcl * P, (xcl + 1) * P
            if layer == 1:
                nc.tensor.matmul(acc[:], lhsT=xc[:, 0, lo:hi], rhs=rhsAB1_sb[:, 0, :], start=True, stop=False)
                nc.tensor.matmul(acc[:], lhsT=xc[:, 1, lo:hi], rhs=rhsAB1_sb[:, 1, :], start=False, stop=False)
                nc.tensor.matmul(acc[:], lhsT=ones_col[:], rhs=brow1_sb[:],
                                 start=False, stop=True, skip_group_check=True)
            else:
                # transpose h1r tile -> lhsT fp16
                hT = hT_pool.tile([P, 2, P], f16, tag="hT")
                for kk in range(2):
                    tp = trps.tile([P, P], f16, space="PSUM", tag="trp")
                    nc.tensor.transpose(tp[:], h_sb[:, t * C + kk * P: t * C + (kk + 1) * P], ident16[:])
                    nc.vector.tensor_copy(hT[:, kk, :], tp[:])
                nc.tensor.matmul(acc[:], lhsT=hT[:, 0, :], rhs=rhsA2_sb[:, 0, :], start=True, stop=False)
                nc.tensor.matmul(acc[:], lhsT=hT[:, 1, :], rhs=rhsA2_sb[:, 1, :], start=False, stop=False)
                nc.tensor.matmul(acc[:, 256:512], lhsT=xc[:, 0, lo:hi], rhs=rhsY2_sb[:, 0, :],
                                 start=False, stop=False, skip_group_check=True)
                nc.tensor.matmul(acc[:, 256:512], lhsT=xc[:, 1, lo:hi], rhs=rhsY2_sb[:, 1, :],
                                 start=False, stop=False, skip_group_check=True)
                nc.tensor.matmul(acc[:], lhsT=ones_col[:], rhs=crow2_sb[:],
                                 start=False, stop=True, skip_group_check=True)
            # hw half -> fp16 slab (scalar engine), z half -> z_sb (vector engine)
            nc.scalar.activation(slab[:, tl * C:(tl + 1) * C], acc[:, 0:256], AF.Copy)
            nc.vector.tensor_copy(z_sb[:, t * C:(t + 1) * C], acc[:, 256:512])

        def flush_chunk(k, slab, agt):
            """DMA chunk slab -> ag bounce; AllGather per chunk (NCHUNK=4) or
            once after the last chunk (NCHUNK=1)."""
            agd, tabs_t = agt
            rows = CHUNK_ROWS[k]
            full_t = rows // P           # full tiles in this chunk
            rem = rows - full_t * P
            r0 = CHUNK_BASE_R[k]
            nc.sync.dma_start(
                agd[r0:r0 + full_t * P, :].rearrange("(t p) c -> p t c", p=P),
                slab[:, 0:full_t * C].rearrange("p (t c) -> p t c", c=C))
            if rem > 0:
                nc.sync.dma_start(agd[r0 + full_t * P:r0 + rows, :],
                                  slab[:rem, full_t * C:(full_t + 1) * C])
            if NOAG:
                return
            if NCHUNK == 1:
                if k == NWIN - 1:
                    nc.gpsimd.collective_compute(
                        "AllGather", mybir.AluOpType.bypass,
                        replica_groups=[list(range(NCORES))],
                        ins=[agd[:].opt()], outs=[tabs_t[:].opt()])
            else:
                nc.gpsimd.collective_compute(
                    "AllGather", mybir.AluOpType.bypass,
                    replica_groups=[list(range(NCORES))],
                    ins=[agd[r0:r0 + rows, :].opt()], outs=[tabs_t[k][:].opt()])

        def gather_st(lay, sti, tabs, idx_sb, sel_in_ap, agd_h):
            """Gathers + selector matmuls for super-tile sti. Returns acc tiles."""
            structure = lay["structure"]
            st_runs = structure[sti]
            st_t0 = sti * ST_TILES
            st_ntiles = min(ST_TILES, TILES - st_t0)
            st_nmm = sum(len(r["mm_list"]) for r in st_runs)
            sel_sb = spool.tile([P, max_mm_per_st * P], f8, tag="s")
            mm_b0 = st_runs[0]["mm_base"]
            nc.sync.dma_start(sel_sb[:, 0:st_nmm * P],
                              sel_in_ap[:, mm_b0 * P:(mm_b0 + st_nmm) * P])
            accs = [cps.tile([P, 512], f32, space="PSUM", tag="agg", name=f"agg{_i}")
                    for _i in range((st_ntiles + 1) // 2)]
            # self-edge term: contiguous reload of this st's own hw rows; one
            # identity matmul per tile replaces 128 gather slots each.
            r0 = st_t0 * P
            rows = min(NPC, r0 + st_ntiles * P) - r0
            full_t = rows // P
            rem = rows - full_t * P
            hw_sb = hwst.tile([P, ST_TILES, C], f16, tag="hwst")
            nc.sync.dma_start(
                hw_sb[:, 0:full_t, :],
                agd_h[r0:r0 + full_t * P, :].rearrange("(t p) c -> p t c", p=P))
            if rem > 0:
                nc.sync.dma_start(hw_sb[0:rem, full_t, :],
                                  agd_h[r0 + full_t * P:r0 + rows, :])
            # mm bookkeeping: global order across runs; first mm per BANK sets
            # start=True (clears whole bank), last mm per TILE sets stop=True.
            # run index -1 = the per-tile identity matmuls (emitted first).
            mm_seq = [(-1, tl, st_t0 + tl) for tl in range(st_ntiles)]
            for ri, run in enumerate(st_runs):
                for (b, ti) in run["mm_list"]:
                    mm_seq.append((ri, b, ti))
            last = {}
            for i, (_, _, ti) in enumerate(mm_seq):
                last[ti] = i
            bank_started = set()
            mm_i = 0
            for (_, tl, ti) in mm_seq[:st_ntiles]:
                kr = P if tl < full_t else rem
                bank = tl // 2
                reg = accs[bank][:, (tl % 2) * 256:(tl % 2) * 256 + 256]
                nc.tensor.matmul(
                    reg, lhsT=ident16[0:kr, :], rhs=hw_sb[0:kr, tl, :],
                    start=(bank not in bank_started), stop=(last[ti] == mm_i),
                    skip_group_check=True)
                bank_started.add(bank)
                mm_i += 1
            for ri, run in enumerate(st_runs):
                ns = run["n_slots"]
                if ns == 0:
                    continue
                w = run["w"]
                sb0 = run["slot_base"]
                run_mms = []
                for (ri2, b, ti) in mm_seq[mm_i:]:
                    if ri2 != ri:
                        break
                    run_mms.append((b, ti))
                # Per-1024-row chunk: small rotating g tiles (4 deep) so the
                # Pool->SDMA->PE chain pipelines at chunk granularity and mms
                # fire as soon as their chunk lands. SWDGE descriptor-ring
                # carveout holds 1024 descriptors; larger gathers wedge.
                GMAX = 1024
                pos = 0
                for g0 in range(0, ns, GMAX):
                    gn = min(GMAX, ns - g0)
                    gb = gn // 128
                    b0 = g0 // 128
                    g_t = gpool.tile([P, (GMAX // P) * C], fm, tag="g")
                    if not NOGATHER:
                        nc.gpsimd.dma_gather(
                            out_ap=g_t[:, 0:gb * C].rearrange("p (b c) -> p b c", c=C),
                            in_ap=tabs[w],
                            idxs_ap=idx_sb[:, (sb0 + g0) // 16:(sb0 + g0 + gn) // 16],
                            num_idxs=gn, num_idxs_reg=gn, elem_size=C,
                        )
                    while pos < len(run_mms) and run_mms[pos][0] < b0 + gb:
                        b, ti = run_mms[pos]
                        tl = ti - st_t0
                        bank = tl // 2
                        reg = accs[bank][:, (tl % 2) * 256:(tl % 2) * 256 + 256]
                        if not NOSEL:
                            si = mm_i - st_ntiles   # sel blob holds slot mms only
                            nc.tensor.matmul(
                                reg, lhsT=sel_sb[:, si * P:(si + 1) * P],
                                rhs=g_t[:, (b - b0) * C:(b - b0 + 1) * C],
                                start=(bank not in bank_started), stop=(last[ti] == mm_i),
                                skip_group_check=True,
                            )
                            bank_started.add(bank)
                        mm_i += 1
                        pos += 1
                assert pos == len(run_mms)
            assert mm_i == len(mm_seq)
            return accs

        def epilogue_st(sti, accs, invdeg_sb):
            """relu(agg*invdeg + z) + LN stats + normalize for super-tile sti."""
            st_t0 = sti * ST_TILES
            st_ntiles = min(ST_TILES, TILES - st_t0)
            for tl in range(st_ntiles):
                t = st_t0 + tl
                reg = accs[tl // 2][:, (tl % 2) * 256:(tl % 2) * 256 + 256]
                tmp = epi.tile([P, C], f32, tag="etmp")
                nc.vector.scalar_tensor_tensor(
                    out=tmp[:], in0=reg, scalar=invdeg_sb[:, t:t + 1],
                    in1=z_sb[:, t * C:(t + 1) * C], op0=OP.mult, op1=OP.add)
                nc.scalar.activation(z_sb[:, t * C:(t + 1) * C], tmp[:], AF.Relu,
                                     accum_out=s1_all[:, t:t + 1])
                sq = epi.tile([P, C], f16, tag="esq")
                nc.scalar.activation(sq[:], z_sb[:, t * C:(t + 1) * C], AF.Square,
                                     accum_out=s2_all[:, t:t + 1])
            # per-st stats: mu = s1/C ; var = s2/C - mu^2 ; rstd = 1/sqrt(var+eps)
            a, b = st_t0, st_t0 + st_ntiles
            nc.vector.tensor_scalar(out=mu_all[:, a:b], in0=s1_all[:, a:b],
                                    scalar1=1.0 / C, scalar2=None, op0=OP.mult)
            var = epi.tile([P, ST_TILES], f32, tag="evar")
            nc.vector.tensor_tensor(out=var[:, 0:b - a], in0=mu_all[:, a:b],
                                    in1=mu_all[:, a:b], op=OP.mult)
            nc.vector.scalar_tensor_tensor(out=var[:, 0:b - a], in0=s2_all[:, a:b],
                                           scalar=1.0 / C, in1=var[:, 0:b - a],
                                           op0=OP.mult, op1=OP.subtract)
            nc.vector.tensor_scalar(out=var[:, 0:b - a], in0=var[:, 0:b - a],
                                    scalar1=float(LN_EPS), scalar2=None, op0=OP.add)
            std = epi.tile([P, ST_TILES], f32, tag="estd")
            nc.scalar.activation(std[:, 0:b - a], var[:, 0:b - a], AF.Sqrt)
            nc.vector.reciprocal(rstd_all[:, a:b], std[:, 0:b - a])
            # normalize -> h_sb fp16
            for tl in range(st_ntiles):
                t = st_t0 + tl
                nc.vector.tensor_scalar(
                    out=h_sb[:, t * C:(t + 1) * C], in0=z_sb[:, t * C:(t + 1) * C],
                    scalar1=mu_all[:, t:t + 1], scalar2=rstd_all[:, t:t + 1],
                    op0=OP.subtract, op1=OP.mult)

        def final_tiles(t0, t1):
            """Project h2 tiles [t0,t1) -> out (DMA straight from PSUM)."""
            for t in range(t0, t1):
                hT = hT_pool.tile([P, 2, P], f16, tag="hT", name="fhT")
                for kk in range(2):
                    tp = trps.tile([P, P], f16, space="PSUM", tag="trp", name="ftp")
                    nc.tensor.transpose(tp[:], h_sb[:, t * C + kk * P: t * C + (kk + 1) * P], ident16[:])
                    nc.vector.tensor_copy(hT[:, kk, :], tp[:])
                acc = abps.tile([P, OUT], f32, space="PSUM", tag="acc", name="facc")
                nc.tensor.matmul(acc[:], lhsT=hT[:, 0, :], rhs=rhsF_sb[:, 0, :], start=True, stop=False)
                nc.tensor.matmul(acc[:], lhsT=hT[:, 1, :], rhs=rhsF_sb[:, 1, :], start=False, stop=False)
                nc.tensor.matmul(acc[:], lhsT=ones_col[:], rhs=browF_sb[:],
                                 start=False, stop=True, skip_group_check=True)
                o_sb = epi.tile([P, OUT], f32, tag="fo")
                nc.vector.tensor_copy(o_sb[:], acc[:])
                nc.sync.dma_start(out_dram[t * P:(t + 1) * P, :], o_sb[:])

        # ---------------- layer 1 dense, chunked AllGather
        for _rep in range(REPEAT):
          for k in range(NWIN):
            slab = slab_pool.tile([P, CHUNK_TILES[k] * C], fm, tag="slab")
            xc = None
            for tl in range(CHUNK_TILES[k]):
                if tl % XC_T == 0:
                    nt = min(XC_T, CHUNK_TILES[k] - tl)
                    xc = load_xc(CHUNK_T0[k] + tl, nt, f"xc1_{k}_{tl}")
                dense_tile(1, CHUNK_T0[k] + tl, slab, tl, xc, tl % XC_T)
            flush_chunk(k, slab, agt1)

          def dense2_range(lo, hi, state):
              """Dense-2 for tiles [lo,hi); fires AG2 chunks as they complete."""
              slab2 = state.get("slab2")
              xc2 = state.get("xc2")
              for t in range(lo, hi):
                  k = min(t // 25, 3)
                  tl = t - CHUNK_T0[k]
                  if slab2 is None:
                      slab2 = slab_pool.tile([P, CHUNK_TILES[k] * C], fm, tag="slab",
                                             name=f"slab2_{k}")
                  if tl % XC_T == 0:
                      nt = min(XC_T, CHUNK_TILES[k] - tl)
                      xc2 = load_xc(t, nt, f"xc2_{k}_{tl}")
                  dense_tile(2, t, slab2, tl, xc2, tl % XC_T)
                  if t == CHUNK_T0[k] + CHUNK_TILES[k] - 1:
                      flush_chunk(k, slab2, agt2)
                      slab2 = None
              state["slab2"] = slab2
              state["xc2"] = xc2

          if INTERLEAVE:
              # ---------------- gather1 with interleaved dense2 + chunked AG2
              d2state = {}
              dense2_done = 0
              for s in range(NST):
                  accs = gather_st(lay1, s, tab1, idx1_sb, sel1_in, ag1)
                  epilogue_st(s, accs, invdeg1_sb)
                  hi = min((s + 1) * ST_TILES, TILES)
                  dense2_range(dense2_done, hi, d2state)
                  dense2_done = hi
              # ---------------- gather2 with interleaved final projection
              for s in range(NST):
                  accs = gather_st(lay2, s, tab2, idx2_sb, sel2_in, ag2)
                  epilogue_st(s, accs, invdeg2_sb)
                  final_tiles(s * ST_TILES, min((s + 1) * ST_TILES, TILES))
          else:
              for s in range(NST):
                  accs = gather_st(lay1, s, tab1, idx1_sb, sel1_in, ag1)
                  epilogue_st(s, accs, invdeg1_sb)
              dense2_range(0, TILES, {})
              for s in range(NST):
                  accs = gather_st(lay2, s, tab2, idx2_sb, sel2_in, ag2)
                  epilogue_st(s, accs, invdeg2_sb)
              final_tiles(0, TILES)

    nc.compile()
    return nc


# ---------------------------------------------------------------- entry point
def kernel(**inputs):
    from concourse.bass_utils import run_bass_kernel_spmd

    in_maps, lay1, lay2 = _prep(inputs)
    key = "nc"
    if key not in _COMPILED:
        _COMPILED[key] = _build_nc(lay1, lay2)
    nc = _COMPILED[key]
    res = run_bass_kernel_spmd(nc, in_maps, core_ids=list(range(NCORES)))
    _COMPILED["last_res"] = res
    out = np.concatenate([res.results[c]["out"][:NPC] for c in range(NCORES)], axis=0)
    return out.astype(np.float32)

